# revision 32
# baseline (speedup 1.0000x reference)
"""Trainium2 Bass kernel for nn_LocalTransformer (4-layer transformer,
d=1024, 16 heads, dff=4096, seq=1024, batch=4, mask: allowed iff j <= i+64).

Sharding: 8 cores = 4 samples x 2 sequence halves; each core owns 512
tokens. Attention context is a relative window of 1152 positions
(p = t - qoff + 512); out-of-window positions are killed by per-core pad
biases added inside exp and an affine_select on the boundary chunks.

K/V for remote positions move via a pairwise AllGather per layer. The
receiver DMAs the rank-0 block into window [0:512] and the rank-1 block's
first 64 columns into window [1024:1088] on BOTH cores - wherever that
placement is wrong, the mask biases already zero the probabilities, so no
fixup arithmetic is needed. V runs (and gathers) first so both gathers
complete before attention needs remote chunks.

Weights are bf16, host-side pre-packed into panel-contiguous layout so
every weight DMA moves >=2KB contiguous per partition. The residual
stream stays fp32 (X32); a bf16 copy (X) feeds the matmuls. Exp is fused
over chunk pairs (equal pad bias within a pair). FFN runs in two
dff-halves with 16-matmul PSUM accumulation groups.
"""
import numpy as np

L, D, H, DFF, S, B = 4, 1024, 16, 4096, 1024, 4
HD = D // H  # 64
T = 512  # local tokens per core
WIN = 1152  # kv window positions (9 chunks of 128)
NC = 9
EPS = 1e-5
NEG = -30000.0
V_E = H * 65  # 1040: per head [V(64) | denominator-ones col]
NPAR = 96  # packed per-layer param columns

_CACHE = {}


def _build_program():
    import concourse.bass as bass
    import concourse.tile as tile
    from concourse import bacc, mybir
    from contextlib import ExitStack

    f32, bf16, f32r = mybir.dt.float32, mybir.dt.bfloat16, mybir.dt.float32r
    AF = mybir.ActivationFunctionType
    ALU = mybir.AluOpType

    nc = bacc.Bacc("TRN2", target_bir_lowering=False, debug=False, num_devices=8)

    I = {}
    I["x0"] = nc.dram_tensor("x0", [D, T], f32r, kind="ExternalInput").ap()
    I["pb"] = nc.dram_tensor("pb", [NC, 128, 1], f32, kind="ExternalInput").ap()
    I["onesr"] = nc.dram_tensor("onesr", [1, T], bf16, kind="ExternalInput").ap()
    I["ones1"] = nc.dram_tensor("ones1", [1, 128], f32r, kind="ExternalInput").ap()
    I["onesd"] = nc.dram_tensor("onesd", [128, 1], f32r, kind="ExternalInput").ap()
    I["par"] = nc.dram_tensor("par", [L, 128, NPAR], f32, kind="ExternalInput").ap()
    for k, sh in (
        ("wq", [L, 8, 128, 1024]),
        ("wk", [L, 8, 128, 1024]),
        ("wo", [L, 8, 128, 1024]),
        ("w1", [L, 32, 128, 1024]),
        ("w2", [L, 8, 128, 4096]),
        ("wv", [L, 8, 128, V_E]),
        ("vbias", [L, 1, V_E]),
    ):
        I[k] = nc.dram_tensor(k, sh, bf16, kind="ExternalInput").ap()
    y = nc.dram_tensor("y", [D, T], f32, kind="ExternalOutput").ap()

    cck_in, cck_out, ccv_in, ccv_out = [], [], [], []
    for l in range(L):
        cck_in.append(nc.dram_tensor(f"ccki{l}", [D, T], bf16, kind="Internal").ap())
        cck_out.append(nc.dram_tensor(f"ccko{l}", [2 * D, T], bf16, kind="Internal").ap())
        ccv_in.append(nc.dram_tensor(f"ccvi{l}", [T, V_E], bf16, kind="Internal").ap())
        ccv_out.append(nc.dram_tensor(f"ccvo{l}", [2 * T, V_E], bf16, kind="Internal").ap())

    RG = [[0, 1], [2, 3], [4, 5], [6, 7]]
    SELEXT = {4: 64, 5: 192, 6: 320, 7: 448, 8: 512}
    CHUNK_GROUPS = [(4, 5), (6, 7), (0, 1), (2, 3), (8,)]

    with tile.TileContext(nc) as tc, ExitStack() as ctx:
        pers = ctx.enter_context(tc.tile_pool(name="pers", bufs=1))
        X = [pers.tile([128, T], bf16, tag=f"X{i}", name=f"X{i}") for i in range(8)]
        X32 = [pers.tile([128, T], f32, tag=f"X32{i}", name=f"X32{i}") for i in range(8)]
        X2 = [pers.tile([128, T], f32r, tag=f"X2{i}", name=f"X2{i}") for i in range(8)]
        OP = [pers.tile([128, T], bf16, tag=f"OP{i}", name=f"OP{i}") for i in range(8)]
        Q = [pers.tile([128, T], bf16, tag=f"Q{i}", name=f"Qt{i}") for i in range(8)]
        KH = pers.tile([128, 8, WIN], bf16, tag="KH", name="KHt")
        VT = pers.tile([128, NC, V_E], bf16, tag="VT", name="VTt")
        FB = [pers.tile([128, T], bf16, tag=f"FB{i}", name=f"FBt{i}") for i in range(8)]
        P1 = pers.tile([65, 16, T], bf16, tag="P1", name="P1t")
        ones_row = pers.tile([1, T], bf16, tag="ones_row", name="ones_row")
        ones1 = pers.tile([1, 128], f32r, tag="ones1", name="ones1t")
        onesd_t = pers.tile([128, 1], f32r, tag="onesd", name="onesdt")
        pb_t = [pers.tile([128, 1], f32, tag=f"pb{i}", name=f"pbt{i}") for i in range(NC)]

        wp = ctx.enter_context(tc.tile_pool(name="wp", bufs=6))  # [128,8,128] bf16
        vp = ctx.enter_context(tc.tile_pool(name="vp", bufs=8))  # [128,1040] bf16
        smw = ctx.enter_context(tc.tile_pool(name="smw", bufs=1))  # vbias
        otp = ctx.enter_context(tc.tile_pool(name="otp", bufs=2))  # [128,T] bf16
        parp = ctx.enter_context(tc.tile_pool(name="parp", bufs=2))  # [128,NPAR]
        pp = ctx.enter_context(tc.tile_pool(name="pp", bufs=2, space="PSUM"))
        scp = ctx.enter_context(tc.tile_pool(name="scp", bufs=2, space="PSUM"))
        pav = ctx.enter_context(tc.tile_pool(name="pav", bufs=2, space="PSUM"))
        pr = ctx.enter_context(tc.tile_pool(name="pr", bufs=4))  # [128,1024] bf16
        avsp = ctx.enter_context(tc.tile_pool(name="avsp", bufs=6))  # [65,T] f32
        tps = ctx.enter_context(tc.tile_pool(name="tps", bufs=4))  # [128,T] f32
        sm = ctx.enter_context(tc.tile_pool(name="sm", bufs=3))  # [1,T] smalls
        rhp = ctx.enter_context(tc.tile_pool(name="rhp", bufs=2))  # [1,T] recip

        r = lambda ap: ap.bitcast(f32r)
        q32 = lambda ap: ap.bitcast(f32)

        def warm(n):
            # dependency-free matmuls that run during engine-wait windows so
            # the PE's HAM clock gate never sees a contiguous idle window;
            # rotate disjoint output rows so Tile adds no WAW serialization
            dm = pp.tile([128, T], f32, tag="ps", name="dums")
            for i in range(n):
                j = (i % 3) * 32
                nc.tensor.matmul(
                    out=dm[j : j + 1, :], lhsT=ones_row[:, 0:1], rhs=ones_row[:],
                    start=True, stop=True,
                )

        nc.sync.dma_start(out=ones_row[:], in_=I["onesr"][:])
        nc.sync.dma_start(out=ones1[:], in_=I["ones1"][:])
        nc.sync.dma_start(out=onesd_t[:], in_=I["onesd"][:])
        for i in range(NC):
            nc.sync.dma_start(out=pb_t[i][:], in_=I["pb"][i])
        for i in range(8):
            nc.sync.dma_start(
                out=X32[i][:], in_=q32(I["x0"][i * 128 : (i + 1) * 128, :])
            )
            nc.vector.tensor_copy(X[i][:], X32[i][:])
        nc.gpsimd.memset(KH[:], 0.0)
        nc.gpsimd.memset(VT[:], 0.0)

        for l in range(L):
            PAR = parp.tile([128, NPAR], f32, tag="par", name="par")
            nc.sync.dma_start(out=PAR[:], in_=I["par"][l])
            p_bq = lambda ec: PAR[:, ec : ec + 1]
            p_bk = lambda ec: PAR[:, 8 + ec : 9 + ec]
            p_bo = lambda ec: PAR[:, 16 + ec : 17 + ec]
            p_b2 = lambda ec: PAR[:, 24 + ec : 25 + ec]
            p_l1g = lambda dc: PAR[:, 32 + dc : 33 + dc]
            p_l1b = lambda dc: PAR[:, 40 + dc : 41 + dc]
            p_l2g = lambda dc: PAR[:, 48 + dc : 49 + dc]
            p_l2b = lambda dc: PAR[:, 56 + dc : 57 + dc]
            p_b1 = lambda fc: PAR[:, 64 + fc : 65 + fc]

            # ---------- K projection -> KH[:, :, 512:1024], stage, AllGather ----------
            for ec in range(8):
                pan = wp.tile([128, 8, 128], bf16, tag="wpan", name="wpan")
                nc.sync.dma_start(
                    out=pan[:], in_=I["wk"][l, ec].rearrange("p (dc c) -> p dc c", c=128)
                )
                ps = pp.tile([128, T], f32, tag="ps", name="ps")
                for dc in range(8):
                    nc.tensor.matmul(
                        out=ps[:], lhsT=pan[:, dc, :], rhs=X[dc][:],
                        start=(dc == 0), stop=(dc == 7),
                    )
                nc.scalar.activation(
                    KH[:, ec, 512:1024], ps[:], AF.Identity, bias=p_bk(ec), scale=1.0
                )
            nc.scalar.dma_start(
                out=cck_in[l][:].rearrange("(ec p) c -> p ec c", p=128),
                in_=KH[:, :, 512:1024],
            )
            nc.gpsimd.collective_compute(
                "AllGather", mybir.AluOpType.bypass, replica_groups=RG,
                ins=[cck_in[l][:]], outs=[cck_out[l][:]],
            )
            # remote K placement (sync queue: does not sit behind AG-V wait)
            nc.sync.dma_start(
                out=KH[:, :, 0:512],
                in_=cck_out[l][0:D, :].rearrange("(ec p) c -> p ec c", p=128),
            )
            nc.sync.dma_start(
                out=KH[:, :, 1024:1088],
                in_=cck_out[l][D : 2 * D, 0:64].rearrange("(ec p) c -> p ec c", p=128),
            )

            # ---------- V projection -> VT[:, 4:8, :], stage, AllGather ----------
            vps = []
            for dc in range(8):
                vt_ = vp.tile([128, V_E], bf16, tag="vpan", name="vpan")
                nc.sync.dma_start(out=vt_[:], in_=I["wv"][l, dc])
                vps.append(vt_)
            vb = smw.tile([1, V_E], bf16, tag="vbias", name="vbias")
            nc.sync.dma_start(out=vb[:], in_=I["vbias"][l])
            for tcx in range(4):
                ps0 = pav.tile([128, T], f32, tag="av", name="vps0")
                ps1 = pav.tile([128, T], f32, tag="av", name="vps1")
                psB = pp.tile([128, T], f32, tag="ps", name="ps")
                for dc in range(8):
                    lx = X[dc][:, tcx * 128 : (tcx + 1) * 128]
                    nc.tensor.matmul(
                        out=ps0[:], lhsT=lx, rhs=vps[dc][:, 0:512],
                        start=(dc == 0), stop=False,
                    )
                    nc.tensor.matmul(
                        out=ps1[:], lhsT=lx, rhs=vps[dc][:, 512:1024],
                        start=(dc == 0), stop=False,
                    )
                    nc.tensor.matmul(
                        out=psB[:, 0:16], lhsT=lx, rhs=vps[dc][:, 1024:1040],
                        start=(dc == 0), stop=False,
                    )
                o1 = ones_row[:, 0:128]
                nc.tensor.matmul(
                    out=ps0[:], lhsT=o1, rhs=vb[:, 0:512], start=False, stop=True
                )
                nc.tensor.matmul(
                    out=ps1[:], lhsT=o1, rhs=vb[:, 512:1024], start=False, stop=True
                )
                nc.tensor.matmul(
                    out=psB[:, 0:16], lhsT=o1, rhs=vb[:, 1024:1040],
                    start=False, stop=True,
                )
                nc.vector.tensor_copy(VT[:, 4 + tcx, 0:512], ps0[:])
                nc.vector.tensor_copy(VT[:, 4 + tcx, 512:1024], ps1[:])
                nc.vector.tensor_copy(VT[:, 4 + tcx, 1024:1040], psB[:, 0:16])
            nc.scalar.dma_start(
                out=ccv_in[l][:].rearrange("(t p) v -> p t v", p=128),
                in_=VT[:, 4:8, :],
            )
            nc.gpsimd.collective_compute(
                "AllGather", mybir.AluOpType.bypass, replica_groups=RG,
                ins=[ccv_in[l][:]], outs=[ccv_out[l][:]],
            )
            # remote V placement (wrong-for-this-core slots are masked)
            nc.sync.dma_start(
                out=VT[:, 0:4, :],
                in_=ccv_out[l][0:T, :].rearrange("(t p) v -> p t v", p=128),
            )
            nc.sync.dma_start(out=VT[0:64, 8, :], in_=ccv_out[l][T : T + 64, :])

            # ---------- Q projection ----------
            for ec in range(8):
                pan = wp.tile([128, 8, 128], bf16, tag="wpan", name="wpan")
                nc.sync.dma_start(
                    out=pan[:], in_=I["wq"][l, ec].rearrange("p (dc c) -> p dc c", c=128)
                )
                ps = pp.tile([128, T], f32, tag="ps", name="ps")
                for dc in range(8):
                    nc.tensor.matmul(
                        out=ps[:], lhsT=pan[:, dc, :], rhs=X[dc][:],
                        start=(dc == 0), stop=(dc == 7),
                    )
                nc.scalar.activation(
                    Q[ec][:], ps[:], AF.Identity, bias=p_bq(ec), scale=1.0
                )

            # ---------- attention, two passes: local chunks for all head
            # pairs first (partials to SBUF bf16), then remote chunks once
            # the AllGathers have landed. Division deferred one head pair.
            avs_of = {}
            dn_of = {}

            def division(kc):
                dn = dn_of[kc]
                rp = rhp.tile([2, T], f32, tag="rp", name="rp")
                nc.vector.reciprocal_approx_fast(rp[:], dn[:])
                for par in range(2):
                    h = 2 * kc + par
                    rht = rhp.tile([1, T], f32r, tag="rh", name="rht")
                    nc.sync.dma_start(out=rht[:], in_=rp[par : par + 1, :].bitcast(f32r))
                    bc = pp.tile([128, T], f32, tag="ps", name="ps")
                    nc.tensor.matmul(
                        out=bc[0:64, :], lhsT=r(ones1[:, 0:64]), rhs=r(rht[:]),
                        start=True, stop=True,
                    )
                    bcs = tps.tile([128, T], f32, tag="t512", name="t512")
                    nc.vector.tensor_copy(bcs[0:64, :], bc[0:64, :])
                    if par == 0:
                        nc.vector.tensor_mul(
                            OP[kc][0:64, :], avs_of[h][0:64, :], bcs[0:64, :]
                        )
                    else:
                        ot = otp.tile([128, T], bf16, tag="otb", name="otr")
                        nc.vector.tensor_mul(
                            ot[0:64, :], avs_of[h][0:64, :], bcs[0:64, :]
                        )
                        nc.gpsimd.dma_start(out=OP[kc][64:128, :], in_=ot[0:64, :])

            def attn_pass(kc, groups, nchunks, final):
                h0 = 2 * kc
                av = [pav.tile([128, T], f32, tag="av", name="av") for _ in range(2)]
                pend = None
                nav = [0]

                def emit_av(pend):
                    cg, ppt = pend
                    for ci, c in enumerate(cg):
                        for par in range(2):
                            nc.tensor.matmul(
                                out=av[par][0:65, :],
                                lhsT=VT[:, c, (h0 + par) * 65 : (h0 + par) * 65 + 65],
                                rhs=ppt[par][:, ci * 512 : (ci + 1) * 512],
                                start=(nav[0] == 0), stop=(nav[0] == nchunks - 1),
                            )
                        nav[0] += 1

                for cg in groups:
                    w = len(cg) * 512
                    sc = [
                        scp.tile([128, 1024], f32, tag="sc", name="sc")
                        for _ in range(2)
                    ]
                    for ci, c in enumerate(cg):
                        for par in range(2):
                            rows = slice(par * 64, par * 64 + 64)
                            nc.tensor.matmul(
                                out=sc[par][:, ci * 512 : (ci + 1) * 512],
                                lhsT=KH[rows, kc, c * 128 : (c + 1) * 128],
                                rhs=Q[kc][rows, :],
                                start=True, stop=True,
                            )
                    if pend is not None:
                        emit_av(pend)
                    pt = [
                        pr.tile([128, 1024], bf16, tag="probs", name="probs")
                        for _ in range(2)
                    ]
                    for par in range(2):
                        nc.scalar.activation(
                            pt[par][:, 0:w], sc[par][:, 0:w], AF.Exp,
                            bias=pb_t[cg[0]][:], scale=1.0,
                        )
                    for ci, c in enumerate(cg):
                        if c in SELEXT:
                            ext = SELEXT[c]
                            off = ci * 512
                            for par in range(2):
                                nc.gpsimd.affine_select(
                                    out=pt[par][:, off : off + ext],
                                    in_=pt[par][:, off : off + ext],
                                    pattern=[[1, ext]], compare_op=ALU.is_ge,
                                    fill=0.0, base=576 - c * 128,
                                    channel_multiplier=-1,
                                )
                    pend = (cg, pt)
                emit_av(pend)
                for par in range(2):
                    h = 2 * kc + par
                    if not final:
                        nc.vector.tensor_copy(P1[:, h, :], av[par][0:65, :])
                    else:
                        a_s = avsp.tile([65, T], f32, tag="avs", name="avs")
                        nc.vector.tensor_add(a_s[:], av[par][0:65, :], P1[:, h, :])
                        if par == 0:
                            dn_of[kc] = rhp.tile([2, T], f32, tag="dn", name="dn")
                        nc.scalar.dma_start(
                            out=dn_of[kc][par : par + 1, :], in_=a_s[64:65, :]
                        )
                        avs_of[h] = a_s

            for kc in range(8):
                attn_pass(kc, [(4, 5), (6, 7)], 4, False)
            for kc in range(8):
                attn_pass(kc, [(0, 1), (2, 3), (8,)], 5, True)
                if kc > 0:
                    division(kc - 1)
            division(7)

            def layernorm(src, pg, pb_, need_bf=True):
                mu = pp.tile([1, T], f32, tag="ps", name="ps")
                ms = pp.tile([1, T], f32, tag="ps", name="ps")
                for dc in range(8):
                    sq = tps.tile([128, T], f32r, tag="t512", name="sqr")
                    nc.vector.tensor_mul(sq[:], q32(src[dc][:]), q32(src[dc][:]))
                    nc.tensor.matmul(
                        out=mu[:], lhsT=r(onesd_t[:]), rhs=r(src[dc][:]),
                        start=(dc == 0), stop=(dc == 7),
                    )
                    nc.tensor.matmul(
                        out=ms[:], lhsT=r(onesd_t[:]), rhs=sq[:],
                        start=(dc == 0), stop=(dc == 7),
                    )
                mu_sb = sm.tile([1, T], f32r, tag="sm1", name="mu")
                nc.vector.tensor_copy(mu_sb[:], mu[:])
                t2 = sm.tile([1, T], f32, tag="sm1", name="t2")
                nc.vector.tensor_mul(t2[:], q32(mu_sb[:]), q32(mu_sb[:]))
                var = sm.tile([1, T], f32, tag="sm1", name="var")
                nc.vector.tensor_sub(var[:], ms[:], t2[:])
                nc.vector.tensor_scalar_add(var[:], var[:], EPS)
                std = sm.tile([1, T], f32, tag="sm1", name="std")
                nc.scalar.sqrt(std[:], var[:])
                rstd32 = sm.tile([1, T], f32, tag="sm1", name="rstd32")
                nc.vector.reciprocal_approx_fast(rstd32[:], std[:])
                rstd = sm.tile([1, T], f32r, tag="sm1", name="rstd")
                nc.vector.tensor_copy(rstd[:], rstd32[:])
                mub = pp.tile([128, T], f32, tag="ps", name="ps")
                nc.tensor.matmul(
                    out=mub[:], lhsT=r(ones1[:]), rhs=r(mu_sb[:]), start=True, stop=True
                )
                rsb = pp.tile([128, T], f32, tag="ps", name="ps")
                nc.tensor.matmul(
                    out=rsb[:], lhsT=r(ones1[:]), rhs=rstd[:], start=True, stop=True
                )
                rsb_sb = tps.tile([128, T], f32, tag="t512", name="t512")
                nc.vector.tensor_copy(rsb_sb[:], rsb[:])
                for dc in range(8):
                    t = tps.tile([128, T], f32, tag="t512", name="t512")
                    nc.vector.tensor_sub(t[:], q32(src[dc][:]), mub[:])
                    t2b = tps.tile([128, T], f32, tag="t512", name="t512")
                    nc.vector.tensor_mul(t2b[:], t[:], rsb_sb[:])
                    nc.scalar.activation(
                        X32[dc][:], t2b[:], AF.Identity, bias=pb_(dc), scale=pg(dc)
                    )
                    if need_bf:
                        nc.vector.tensor_copy(X[dc][:], X32[dc][:])

            # ---------- Wo + residual + LN1 ----------
            for ec in range(8):
                pan = wp.tile([128, 8, 128], bf16, tag="wpan", name="wpan")
                nc.sync.dma_start(
                    out=pan[:], in_=I["wo"][l, ec].rearrange("p (dc c) -> p dc c", c=128)
                )
                ps = pp.tile([128, T], f32, tag="ps", name="ps")
                for dc in range(8):
                    nc.tensor.matmul(
                        out=ps[:], lhsT=pan[:, dc, :], rhs=OP[dc][:],
                        start=(dc == 0), stop=(dc == 7),
                    )
                nc.vector.scalar_tensor_tensor(
                    out=X2[ec][:], in0=ps[:], scalar=p_bo(ec), in1=X32[ec][:],
                    op0=ALU.add, op1=ALU.add,
                )
            layernorm(X2, p_l1g, p_l1b)

            # ---------- FFN in four dff-quarters, W2 accumulated in PSUM ----------
            for quarter in range(4):
                for k in range(8):
                    fc = quarter * 8 + k
                    pan = wp.tile([128, 8, 128], bf16, tag="wpan", name="wpan")
                    nc.sync.dma_start(
                        out=pan[:],
                        in_=I["w1"][l, fc].rearrange("p (dc c) -> p dc c", c=128),
                    )
                    ps = pp.tile([128, T], f32, tag="ps", name="ps")
                    for dc in range(8):
                        nc.tensor.matmul(
                            out=ps[:], lhsT=pan[:, dc, :], rhs=X[dc][:],
                            start=(dc == 0), stop=(dc == 7),
                        )
                    nc.scalar.activation(
                        FB[k][:], ps[:], AF.Relu, bias=p_b1(fc), scale=1.0
                    )
                for ec in range(8):
                    pan = wp.tile([128, 8, 128], bf16, tag="wpan", name="wpan")
                    nc.sync.dma_start(
                        out=pan[:],
                        in_=I["w2"][l, ec][
                            :, quarter * 1024 : (quarter + 1) * 1024
                        ].rearrange("p (k c) -> p k c", c=128),
                    )
                    ps = pp.tile([128, T], f32, tag="ps", name="ps")
                    for k in range(8):
                        nc.tensor.matmul(
                            out=ps[:], lhsT=pan[:, k, :], rhs=FB[k][:],
                            start=(k == 0), stop=(k == 7),
                        )
                    if quarter == 0:
                        nc.vector.scalar_tensor_tensor(
                            out=X2[ec][:], in0=ps[:], scalar=p_b2(ec), in1=X32[ec][:],
                            op0=ALU.add, op1=ALU.add,
                        )
                    else:
                        nc.vector.tensor_add(X2[ec][:], q32(X2[ec][:]), ps[:])
            layernorm(X2, p_l2g, p_l2b, need_bf=(l < L - 1))

        for ec in range(8):
            nc.sync.dma_start(out=y[ec * 128 : (ec + 1) * 128, :], in_=X32[ec][:])

    nc.compile()
    return nc


def _host_prep(inputs):
    import ml_dtypes

    bf = ml_dtypes.bfloat16

    def pack(WT, s=1.0):
        """[Din, Dout] -> [Dout/128, 128, Din] with pan[ec,p,dc*128+c] =
        WT[dc*128+p, ec*128+c]."""
        Din, Dout = WT.shape
        A = (WT * s).reshape(Din // 128, 128, Dout // 128, 128)
        return np.ascontiguousarray(
            A.transpose(2, 1, 0, 3).reshape(Dout // 128, 128, Din)
        ).astype(bf)

    g = {}
    Wqkv = np.asarray(inputs["Wqkv"], np.float32)
    bqkv = np.asarray(inputs["bqkv"], np.float32)
    sc = 1.0 / np.sqrt(HD)
    wq = np.zeros((L, 8, 128, D), bf)
    wk = np.zeros((L, 8, 128, D), bf)
    wo = np.zeros((L, 8, 128, D), bf)
    w1 = np.zeros((L, 32, 128, D), bf)
    w2 = np.zeros((L, 8, 128, DFF), bf)
    wv = np.zeros((L, 8, 128, V_E), bf)
    vbias = np.zeros((L, 1, V_E), bf)
    par = np.zeros((L, 128, NPAR), np.float32)
    for l in range(L):
        Wq, Wk, Wv = Wqkv[l, 0:D], Wqkv[l, D : 2 * D], Wqkv[l, 2 * D :]
        bv = bqkv[l, 2 * D :]
        wq[l] = pack(Wq.T, sc)
        wk[l] = pack(Wk.T)
        wo[l] = pack(np.asarray(inputs["Wo"], np.float32)[l].T)
        w1[l] = pack(np.asarray(inputs["W1"], np.float32)[l].T)
        w2[l] = pack(np.asarray(inputs["W2"], np.float32)[l].T)
        vfull = np.zeros((D, V_E), np.float32)
        vbrow = np.zeros(V_E, np.float32)
        for h in range(H):
            vfull[:, h * 65 : h * 65 + 64] = Wv.T[:, h * 64 : h * 64 + 64]
            vbrow[h * 65 : h * 65 + 64] = bv[h * 64 : h * 64 + 64]
            vbrow[h * 65 + 64] = 1.0  # denominator ones column via bias
        wv[l] = vfull.reshape(8, 128, V_E).astype(bf)
        vbias[l, 0, :] = vbrow.astype(bf)
        # packed per-layer params: [bq, bk, bo, b2, l1g, l1b, l2g, l2b, b1]
        par[l, :, 0:8] = (bqkv[l, 0:D] * sc).reshape(8, 128).T
        par[l, :, 8:16] = bqkv[l, D : 2 * D].reshape(8, 128).T
        par[l, :, 16:24] = np.asarray(inputs["bo"], np.float32)[l].reshape(8, 128).T
        par[l, :, 24:32] = np.asarray(inputs["b2"], np.float32)[l].reshape(8, 128).T
        par[l, :, 32:40] = np.asarray(inputs["g1"], np.float32)[l].reshape(8, 128).T
        par[l, :, 40:48] = np.asarray(inputs["be1"], np.float32)[l].reshape(8, 128).T
        par[l, :, 48:56] = np.asarray(inputs["g2"], np.float32)[l].reshape(8, 128).T
        par[l, :, 56:64] = np.asarray(inputs["be2"], np.float32)[l].reshape(8, 128).T
        par[l, :, 64:96] = np.asarray(inputs["b1"], np.float32)[l].reshape(32, 128).T
    g["wq"], g["wk"], g["wo"], g["w1"], g["w2"], g["wv"], g["vbias"] = (
        wq, wk, wo, w1, w2, wv, vbias,
    )
    g["par"] = par
    g["ones1"] = np.ones((1, 128), np.float32)
    g["onesr"] = np.ones((1, T), bf)
    g["onesd"] = np.full((128, 1), 1.0 / D, np.float32)

    xb = np.asarray(inputs["x"], np.float32).transpose(1, 0, 2)
    in_maps = []
    for c in range(8):
        b, hh = c // 2, c % 2
        pb = np.zeros((NC, 128, 1), np.float32)
        if hh == 0:
            pb[0:4] = NEG
            pb[8, 64:128] = NEG
        else:
            pb[8] = NEG
        m = dict(g)
        m["x0"] = np.ascontiguousarray(xb[b, hh * T : (hh + 1) * T, :].T)
        m["pb"] = pb
        in_maps.append(m)
    return in_maps


def kernel(**inputs):
    from concourse.bass_utils import run_bass_kernel_spmd

    if "nc" not in _CACHE:
        _CACHE["nc"] = _build_program()
    nc = _CACHE["nc"]
    in_maps = _host_prep(inputs)
    res = run_bass_kernel_spmd(nc, in_maps, core_ids=list(range(8)))
    out = np.zeros((S, B, D), np.float32)
    for c in range(8):
        b, hh = c // 2, c % 2
        out[hh * T : (hh + 1) * T, b, :] = res.results[c]["y"].T
    return out
